# revision 6
# baseline (speedup 1.0000x reference)
"""Trainium2 Bass kernel for the 2-layer CIN — square-trick architecture.

Reference computation (per batch element b, embedding channel d):
  z0[hf=h*40+f]  = x[b,h,d] * x[b,f,d]              (h,f in 0..39)
  y0[o]          = relu(sum_hf W0[o,hf,d] * z0[hf] + b0[o])   -> x1[b,o,d]
  z1[hf=h1*40+f] = x1[b,h1,d] * x[b,f,d]            (h1 in 0..63)
  y1[o]          = relu(sum_hf W1[o,hf,d] * z1[hf] + b1[o])   -> x2[b,o,d]
  out[b] = [sum_d x[b,:,d] | sum_d x1[b,:,d] | sum_d x2[b,:,d]]   (2048, 168)

Key identity: x_h*x_f = ((x_h+x_f)^2 - x_h^2 - x_f^2)/2.  The SUM x_h+x_f
comes out of ONE PE matmul with a 2-hot selection stationary (fp32 PSUM);
the square+bf16-cast rides on the ScalarE activation (replacing both the
PSUM->SBUF copy and the DVE multiply); the x^2 terms fold into one small
correction matmul per layer (the weights collapse over f host-side).  A
"V1" K-tile therefore streams through ScalarE only, and is fully free in
how (h,f) pairs map to contraction rows:

- Layer 1 is symmetric (z0[h,f]=z0[f,h]), so V1 tiles cover the 820
  unordered pairs with folded weights W[o,hf]+W[o,fh]: 7 K-tiles instead
  of 14, all V1, no DMA.
- Layer 2 is packed f-major: tile t covers f in {2t,2t+1} x h1 in 0..63 —
  exactly 20 full 128-row tiles.  For DMA-mode tiles the replicated
  factor is then x[f] (STATIC, straight from DRAM xt — no x1 round-trip);
  the resident factor is x1 duplicated into both partition halves of an
  SBUF tile (two cheap DVE copies per group).  "V4" = DVE multiply,
  "V8" = Pool (gpsimd) multiply; MODE1 is the engine-balance tuning knob.

The d-sum accumulations run on Pool; output transposes on PE.  Sharding:
4-way batch x 2-way embedding-channel split (8 cores), host adds the two
d-halves.
"""

from contextlib import ExitStack

import numpy as np
import ml_dtypes

import concourse.bass as bass
import concourse.bacc as bacc
import concourse.tile as tile
from concourse import mybir
from concourse.bass_utils import run_bass_kernel_spmd
from concourse.masks import make_identity

BF16 = mybir.dt.bfloat16
FP32 = mybir.dt.float32
NPBF16 = ml_dtypes.bfloat16
ACT = mybir.ActivationFunctionType

B, F, D = 2048, 40, 32
O0, O1 = 64, 64
NCORES = 8
NB = 4                      # batch shards
ND = 2                      # d shards
BC = B // NB                # 512 batch rows per core
DC = D // ND                # 16 embedding channels per core
H0, H1 = F * F, O0 * F      # 1600, 2560 contraction rows
HS = 120                    # used rows per layer-1 K-tile
KT = 128
NP0 = (F * (F + 1)) // 2    # 820 unordered layer-1 pairs
NT0 = (NP0 + HS - 1) // HS  # 7 folded K-tiles, layer 1 (all V1)
NT1 = H1 // KT              # 20 f-major K-tiles, layer 2 (no padding)
DPG = 2                     # d-channels per group (one PSUM pair-tile)
DG = DC // DPG              # 8 d-groups
NCOL = DPG * BC             # 1024 free columns per group (d-major, b-minor)
NMM = 512                   # fp32-PSUM matmul free size
X1O = 64                    # x1 row offset in the stacked rhs (0-39 x, 64+ x1)

# Layer-2 per-K-tile mode: 0 = V1 (PE sum-matmul + ScalarE square),
# 1 = V4 (DMA-replicated x[f] + DVE mul), 2 = V8 (DMA + Pool mul).
MODE1 = [0,1,0,1,0,1,0,1,0,1,0,1,0,1,0,1,0,0,0,0]
assert len(MODE1) == NT1

L1_PAIRS = [(h, f) for h in range(F) for f in range(h, F)]
assert len(L1_PAIRS) == NP0


def _build_bass(reps=1):
    nc = bacc.Bacc()
    xt = nc.declare_dram_parameter("xt", [F, DC * BC], BF16, isOutput=False)
    w0t = nc.declare_dram_parameter("w0t", [KT, NT0 * DC * O0], BF16, isOutput=False)
    w1t = nc.declare_dram_parameter("w1t", [KT, NT1 * DC * O1], BF16, isOutput=False)
    sel0 = nc.declare_dram_parameter("sel0", [F, NT0 * KT], BF16, isOutput=False)
    sel1 = nc.declare_dram_parameter("sel1", [KT, NT1 * KT], BF16, isOutput=False)
    c1 = nc.declare_dram_parameter("c1", [F, DC * O0], BF16, isOutput=False)
    c2 = nc.declare_dram_parameter("c2", [KT, DC * O1], BF16, isOutput=False)
    b0 = nc.declare_dram_parameter("b0", [O0, 1], FP32, isOutput=False)
    b1 = nc.declare_dram_parameter("b1", [O1, 1], FP32, isOutput=False)
    out = nc.declare_dram_parameter("out", [BC, O0 + O1], FP32, isOutput=True)

    with ExitStack() as ctx:
        tc = ctx.enter_context(tile.TileContext(nc))
        singles = ctx.enter_context(tc.tile_pool(name="singles", bufs=1))
        su_ps = ctx.enter_context(tc.tile_pool(name="su_ps", bufs=2, space="PSUM"))
        y_ps = ctx.enter_context(tc.tile_pool(name="y_ps", bufs=4, space="PSUM"))
        z_sb = ctx.enter_context(tc.tile_pool(name="z_sb", bufs=6))
        xh_sb = ctx.enter_context(tc.tile_pool(name="xh_sb", bufs=6))
        x2_sb = ctx.enter_context(tc.tile_pool(name="x2_sb", bufs=2))
        o_sb = ctx.enter_context(tc.tile_pool(name="o_sb", bufs=2))

        # ---- resident tensors ----
        xstack = singles.tile([KT, DC * BC], BF16)   # 0-39: x, 64-127: x1
        qstack = singles.tile([KT, DC * BC], BF16)   # squares of xstack rows
        x1rep = singles.tile([KT, DC * BC], BF16)
        w0s = singles.tile([KT, NT0, DC * O0], BF16)
        w1s = singles.tile([KT, NT1, DC * O1], BF16)
        sel0s = singles.tile([F, NT0, KT], BF16)
        sel1s = singles.tile([KT, NT1, KT], BF16)
        c1s = singles.tile([F, DC * O0], BF16)
        c2s = singles.tile([KT, DC * O1], BF16)
        b0s = singles.tile([O0, 1], FP32)
        b1s = singles.tile([O1, 1], FP32)

        xt_ap = xt[:]

        def load_inputs():
            nc.gpsimd.dma_start(out=sel0s, in_=sel0[:])
            nc.vector.memset(xstack[32:X1O, :], 0.0)
            nc.vector.memset(qstack[32:X1O, :], 0.0)
            nc.gpsimd.dma_start(out=xstack[0:F, :], in_=xt[:])
            nc.gpsimd.dma_start(out=sel1s, in_=sel1[:])
            nc.gpsimd.dma_start(out=c1s, in_=c1[:])
            nc.gpsimd.dma_start(out=c2s, in_=c2[:])
            nc.vector.memset(zeros64, 0.0)
            nc.gpsimd.dma_start(out=b0s, in_=b0[:])
            nc.gpsimd.dma_start(out=b1s, in_=b1[:])
            nc.sync.dma_start(out=w0s, in_=w0t[:])
            nc.gpsimd.dma_start(out=w1s, in_=w1t[:])
            # static squares of the x rows (corrections contract these)
            nc.scalar.activation(
                out=qstack[0:F, :], in_=xstack[0:F, :], func=ACT.Square
            )

        ident = singles.tile([128, 128], FP32)
        make_identity(nc, ident)

        # split accumulators (even/odd groups) halve the serial
        # read-modify-write chain on Pool; merged once in the epilogue
        acc1 = singles.tile([O0, BC], FP32)
        acc2 = singles.tile([O1, BC], FP32)
        acc1b = singles.tile([O0, BC], FP32)
        acc2b = singles.tile([O1, BC], FP32)
        zeros64 = singles.tile([O1, BC], BF16)

        def layer(g, nt, modes, sels, ws, cs, kc, ksel, kreal, odim,
                  mul_in1, xh_src):
            """One CIN layer for d-group g. Returns the (128, BC) PSUM pair."""
            col0 = g * NCOL
            yp = y_ps.tile([2 * odim, BC], FP32, tag="y", name=f"y_{g}")
            # correction matmuls open the accumulation group
            for i in range(DPG):
                d = g * DPG + i
                nc.tensor.matmul(
                    yp[i * odim:(i + 1) * odim, :],
                    lhsT=cs[0:kc, d * odim:(d + 1) * odim],
                    rhs=qstack[0:kc, col0 + i * BC: col0 + (i + 1) * BC],
                    start=True,
                    stop=False,
                    skip_group_check=True,
                )
            for t in range(nt):
                z = z_sb.tile([KT, NCOL], BF16, tag="z")
                if modes[t] == 0:
                    su = su_ps.tile([KT, NCOL], FP32, tag="su")
                    for h in range(NCOL // NMM):
                        nc.tensor.matmul(
                            su[:, h * NMM:(h + 1) * NMM],
                            lhsT=sels[0:ksel, t, :],
                            rhs=xstack[0:ksel, col0 + h * NMM: col0 + (h + 1) * NMM],
                            start=True,
                            stop=True,
                        )
                    nc.scalar.activation(out=z, in_=su, func=ACT.Square)
                else:
                    xh = xh_sb.tile([KT, NCOL], BF16, tag="xh")
                    nc.sync.dma_start(out=xh, in_=xh_src(t))
                    eng = nc.vector if modes[t] == 1 else nc.gpsimd
                    eng.tensor_mul(
                        z[0:kreal, :], xh[0:kreal, :],
                        mul_in1[0:kreal, col0:col0 + NCOL],
                    )
                for i in range(DPG):
                    d = g * DPG + i
                    nc.tensor.matmul(
                        yp[i * odim:(i + 1) * odim, :],
                        lhsT=ws[0:kreal, t, d * odim:(d + 1) * odim],
                        rhs=z[0:kreal, i * BC:(i + 1) * BC],
                        start=False,
                        stop=(t == nt - 1),
                        skip_group_check=True,
                    )
            return yp

        def layer1(g):
            return layer(g, NT0, [0] * NT0, sel0s, w0s, c1s, F, F, HS, O0,
                         None, None)

        load_inputs()
        for rep in range(reps):
          nc.gpsimd.memset(acc1, 0.0)
          nc.gpsimd.memset(acc2, 0.0)
          nc.gpsimd.memset(acc1b, 0.0)
          nc.gpsimd.memset(acc2b, 0.0)
          yp0 = layer1(0)
          for g in range(DG):
            col0 = g * NCOL
            for i in range(DPG):
                nc.scalar.activation(
                    out=xstack[X1O:KT, col0 + i * BC: col0 + (i + 1) * BC],
                    in_=yp0[i * O0:(i + 1) * O0, :],
                    func=ACT.Relu,
                    bias=b0s,
                    scale=1.0,
                )
            nc.sync.dma_start(
                out=x1rep[0:O0, col0:col0 + NCOL],
                in_=xstack[X1O:KT, col0:col0 + NCOL],
            )
            nc.sync.dma_start(
                out=x1rep[X1O:KT, col0:col0 + NCOL],
                in_=xstack[X1O:KT, col0:col0 + NCOL],
            )
            for i in range(DPG):
                a1 = acc1 if g % 2 == 0 else acc1b
                nc.gpsimd.tensor_add(
                    a1, a1, x1rep[0:O0, col0 + i * BC: col0 + (i + 1) * BC]
                )
            nc.gpsimd.tensor_mul(
                qstack[X1O:KT, col0:col0 + NCOL],
                xstack[X1O:KT, col0:col0 + NCOL],
                xstack[X1O:KT, col0:col0 + NCOL],
            )
            # software pipeline: issue the NEXT group's layer 1 before this
            # group's layer 2, so engine FIFOs never head-of-line block on
            # the relu -> layer-2 dependency (layer 1 reads only x rows).
            yp0_next = layer1(g + 1) if g + 1 < DG else None

            def xh2_src(t):
                return bass.AP(
                    tensor=xt_ap.tensor,
                    offset=xt_ap.offset + 2 * t * DC * BC + col0,
                    ap=[[DC * BC, 2], [0, O0], [1, NCOL]],
                )

            yp1 = layer(g, NT1, MODE1, sel1s, w1s, c2s, KT, KT, KT, O1,
                        x1rep, xh2_src)
            for i in range(DPG):
                x2 = x2_sb.tile([O1, BC], BF16, tag="x2")
                nc.vector.scalar_tensor_tensor(
                    out=x2,
                    in0=yp1[i * O1:(i + 1) * O1, :],
                    scalar=b1s[:],
                    in1=zeros64,
                    op0=mybir.AluOpType.add,
                    op1=mybir.AluOpType.max,
                )
                a2 = acc2 if g % 2 == 0 else acc2b
                nc.gpsimd.tensor_add(a2, a2, x2)
            yp0 = yp0_next

          # ---- epilogue: merge split accumulators, transpose, store ----
          nc.gpsimd.tensor_add(acc1, acc1, acc1b)
          nc.gpsimd.tensor_add(acc2, acc2, acc2b)
          for bh in range(BC // 128):
            outT = o_sb.tile([128, O0 + O1], FP32, tag="outT")
            for acc, off in ((acc1, 0), (acc2, O0)):
                pt = y_ps.tile([128, 64], FP32, tag="y")
                nc.tensor.transpose(
                    pt, acc[:, bh * 128:(bh + 1) * 128], ident[0:64, 0:64]
                )
                nc.vector.tensor_copy(out=outT[:, off:off + 64], in_=pt)
            nc.sync.dma_start(
                out=out[bh * 128:(bh + 1) * 128, :], in_=outT
            )

    nc.compile()
    return nc


_NC_CACHE = {}
LAST_RESULT = None


def _get_nc(reps=1):
    if reps not in _NC_CACHE:
        _NC_CACHE[reps] = _build_bass(reps)
    return _NC_CACHE[reps]


def _l2_pair(t, p):
    """f-major layer-2 packing: tile t, row p -> (h1, f, hf)."""
    f = 2 * t + p // O0
    h1 = p % O0
    return h1, f, h1 * F + f


def _host_prep(x, W0, b0, W1, b1):
    """Build per-core input maps (host-side layout prep, all cheap numpy)."""
    def prep_w0(dh):
        Wd = W0[:, :, dh * DC:(dh + 1) * DC].astype(np.float32)  # (o, 1600, DC)
        tiles = np.zeros((NT0, KT, DC * O0), dtype=NPBF16)
        for t in range(NT0):
            blk = np.zeros((O0, HS, DC), dtype=np.float32)
            for p in range(HS):
                idx = t * HS + p
                if idx >= NP0:
                    break
                h, f = L1_PAIRS[idx]
                w = Wd[:, h * F + f, :]
                if f != h:
                    w = w + Wd[:, f * F + h, :]
                blk[:, p, :] = 0.5 * w
            tiles[t, :HS] = (
                blk.transpose(1, 2, 0).reshape(HS, DC * O0).astype(NPBF16)
            )
        return np.ascontiguousarray(
            tiles.transpose(1, 0, 2).reshape(KT, NT0 * DC * O0)
        )

    def prep_w1(dh):
        Wd = W1[:, :, dh * DC:(dh + 1) * DC].astype(np.float32)  # (o, 2560, DC)
        tiles = np.zeros((NT1, KT, DC * O1), dtype=NPBF16)
        for t in range(NT1):
            scale = 0.5 if MODE1[t] == 0 else 1.0
            blk = np.zeros((O1, KT, DC), dtype=np.float32)
            for p in range(KT):
                _, _, hf = _l2_pair(t, p)
                blk[:, p, :] = scale * Wd[:, hf, :]
            tiles[t] = (
                blk.transpose(1, 2, 0).reshape(KT, DC * O1).astype(NPBF16)
            )
        return np.ascontiguousarray(
            tiles.transpose(1, 0, 2).reshape(KT, NT1 * DC * O1)
        )

    sel0v = np.zeros((F, NT0, KT), dtype=np.float32)
    for t in range(NT0):
        for p in range(HS):
            idx = t * HS + p
            if idx >= NP0:
                break
            h, f = L1_PAIRS[idx]
            sel0v[h, t, p] += 1.0
            sel0v[f, t, p] += 1.0
    sel0v = sel0v.reshape(F, NT0 * KT).astype(NPBF16)

    sel1v = np.zeros((KT, NT1, KT), dtype=np.float32)
    for t in range(NT1):
        if MODE1[t] != 0:
            continue
        for p in range(KT):
            h1, f, _ = _l2_pair(t, p)
            sel1v[X1O + h1, t, p] += 1.0
            sel1v[f, t, p] += 1.0
    sel1v = sel1v.reshape(KT, NT1 * KT).astype(NPBF16)

    def prep_c1(dh):
        Wd = W0[:, :, dh * DC:(dh + 1) * DC].astype(np.float32)
        c = np.zeros((F, DC, O0), dtype=np.float32)
        for (h, f) in L1_PAIRS:
            w = Wd[:, h * F + f, :]
            if f != h:
                w = w + Wd[:, f * F + h, :]
            c[h] -= 0.5 * w.T
            c[f] -= 0.5 * w.T
        return c.reshape(F, DC * O0).astype(NPBF16)

    def prep_c2(dh):
        Wd = W1[:, :, dh * DC:(dh + 1) * DC].astype(np.float32)
        c = np.zeros((KT, DC, O1), dtype=np.float32)
        for t in range(NT1):
            if MODE1[t] != 0:
                continue
            for p in range(KT):
                h1, f, hf = _l2_pair(t, p)
                w = Wd[:, hf, :]
                c[X1O + h1] -= 0.5 * w.T
                c[f] -= 0.5 * w.T
        return c.reshape(KT, DC * O1).astype(NPBF16)

    b0h = b0.reshape(O0, 1).astype(np.float32)
    b1h = b1.reshape(O1, 1).astype(np.float32)

    halves = []
    for dh in range(ND):
        halves.append({
            "w0t": prep_w0(dh),
            "w1t": prep_w1(dh),
            "c1": prep_c1(dh),
            "c2": prep_c2(dh),
        })

    in_maps = []
    for c in range(NCORES):
        bs, dh = c % NB, c // NB
        xc = x[bs * BC:(bs + 1) * BC]                    # (512, 40, 32)
        xtc = np.ascontiguousarray(
            xc[:, :, dh * DC:(dh + 1) * DC].transpose(1, 2, 0).reshape(F, DC * BC)
        ).astype(NPBF16)
        in_maps.append({
            "xt": xtc,
            "sel0": sel0v,
            "sel1": sel1v,
            "b0": b0h,
            "b1": b1h,
            **halves[dh],
        })
    return in_maps


def kernel(x, W0, b0, W1, b1):
    global LAST_RESULT
    x = np.asarray(x, dtype=np.float32)
    W0 = np.asarray(W0, dtype=np.float32)
    W1 = np.asarray(W1, dtype=np.float32)
    b0 = np.asarray(b0, dtype=np.float32)
    b1 = np.asarray(b1, dtype=np.float32)

    nc = _get_nc()
    in_maps = _host_prep(x, W0, b0, W1, b1)
    res = run_bass_kernel_spmd(nc, in_maps, core_ids=list(range(NCORES)))
    LAST_RESULT = res

    out = np.empty((B, F + O0 + O1), dtype=np.float32)
    out[:, :F] = x.sum(axis=-1)
    for bs in range(NB):
        half0 = np.asarray(res.results[bs]["out"])
        half1 = np.asarray(res.results[NB + bs]["out"])
        out[bs * BC:(bs + 1) * BC, F:] = half0 + half1
    return out
